# revision 4
# baseline (speedup 1.0000x reference)
# BasisConvLayer forward on 8 TRN2 NeuronCores — V2.
#
# Per-core strategy (cores shard by destination row range, 12500 rows each):
# x is staged in SBUF as 8 transposed blocks of 12512 nodes ([16 comps, n]
# per 16-partition band). Edges are bucketed by source block (band) and
# dest-sorted within each band. An 8-way-parallel Q7 ap_gather fetches x_j
# per edge into [16, e] strips. PE expands x_j to the 64-row (f,u) grid and
# contracts with W2[(f,u),(v,o)] (dense 16-term basis product, bu/bv hat
# values streamed from DRAM as bf16); DVE applies bu/bv elementwise and
# folds v, yielding per-band msg strips [16, e]. Destination segment-sum is
# a DVE prefix scan along each band strip plus a small ap_gather of per-row
# boundary prefixes; diffs + cross-band partition folds give [16, 12500]
# per core. No DMA scatter/gather descriptors per edge at all.
import sys
import numpy as np

sys.path.insert(0, '/opt/trn_rl_repo')

N_NODES = 100000
N_EDGES = 1600000
F = 16
NB = 4
N_CORES = 8
ROWS_PER_CORE = N_NODES // N_CORES      # 12500
BLK = 12512                              # nodes per band block
P = 128
CH = 512                                 # gather-chunk slots per band
SUB = 512                                # psum sub-chunk
RCH = 800                                # boundary entries per chunk
N_RCH = 16                               # 16*799 >= 12500 rows


def _linear_basis(u, n=4):
    centers = np.linspace(-1.0, 1.0, n, dtype=np.float32)
    dx = 2.0 / (n - 1)
    return np.maximum(0.0, 1.0 - np.abs(u[:, None] - centers[None, :]) / dx)


def _host_prep(x, edge_index, edge_attr, weight):
    import ml_dtypes
    bf16 = ml_dtypes.bfloat16
    x = np.asarray(x, np.float32)
    ei = np.asarray(edge_index, np.int64)
    ea = np.asarray(edge_attr, np.float32)
    w = np.asarray(weight, np.float32)

    row, col = ei[0], ei[1]
    bu = _linear_basis(ea[:, 0])            # [E, 4]
    bv = _linear_basis(ea[:, 1])            # [E, 4]
    core = row // ROWS_PER_CORE
    row_loc = row - core * ROWS_PER_CORE
    band = col // BLK
    idx16 = (col - band * BLK).astype(np.int16)

    # order: (core, band, row_loc)
    order = np.lexsort((row_loc, band, core))
    core_s = core[order]
    band_s = band[order]
    row_s = row_loc[order]
    idx_s = idx16[order]
    bu_s = bu[order]
    bv_s = bv[order]

    # counts per (core, band)
    cb = core_s * 8 + band_s
    counts = np.bincount(cb, minlength=64).reshape(N_CORES, 8)
    Tb = int(counts.max()) + 1
    Tb = ((Tb + CH - 1) // CH) * CH          # multiple of CH
    n_chunks = Tb // CH

    # xsrc [128, BLK]: partition 16b+j , col n -> x[BLK*b+n, j]
    xpad = np.zeros((8 * BLK, F), np.float32)
    xpad[:N_NODES] = x
    xsrc = np.ascontiguousarray(
        xpad.reshape(8, BLK, F).transpose(0, 2, 1).reshape(P, BLK))

    # W2 [64,64]: [(f,u),(v,o)] = w[u,v,f,o]; W2D = two copies
    W2 = w.transpose(2, 0, 1, 3).reshape(F, NB, NB * F)  # [f, u, (v,o)]
    W2 = W2.reshape(F * NB, NB * F)                      # [(f,u), (v,o)]
    W2D = np.concatenate([W2, W2], axis=0).astype(bf16)  # [128, 64]

    # E32 [32, 128]: E32[bs*16+f, bs*64+f*4+u] = 1 ; E32D copies at 32q
    E32 = np.zeros((32, 128), np.float32)
    for bs in range(2):
        for f in range(F):
            for u in range(NB):
                E32[bs * 16 + f, bs * 64 + f * 4 + u] = 1.0
    E32D = np.tile(E32, (4, 1)).astype(bf16)             # [128, 128]

    # Fv2 [128, 32]: rows (half*64 + v*16 + o) -> col half*16 + o
    Fv2 = np.zeros((128, 32), np.float32)
    for half in range(2):
        for v in range(NB):
            for o in range(F):
                Fv2[half * 64 + v * 16 + o, half * 16 + o] = 1.0
    Fv2 = Fv2.astype(bf16)
    # Fsum [128, 16]: rows (16b+o) -> col o
    Fsum = np.zeros((128, 16), np.float32)
    for b in range(8):
        for o in range(F):
            Fsum[16 * b + o, o] = 1.0
    Fsum = Fsum.astype(bf16)

    def wrap16(a, width):
        # a: [8, width] -> [128, width//16] partition-wrapped per band
        out = np.zeros((P, width // 16), a.dtype)
        for b in range(8):
            out[16 * b:16 * (b + 1)] = a[b].reshape(width // 16, 16).T
        return out

    in_maps = []
    for c in range(N_CORES):
        m = core_s == c
        bnd = band_s[m]
        rl = row_s[m].astype(np.int64)
        ix = idx_s[m]
        buc = bu_s[m]
        bvc = bv_s[m]

        gidx = np.zeros((8, Tb), np.int16)
        buE = np.zeros((8, Tb, 4), np.float32)
        bvE = np.zeros((8, Tb, 4), np.float32)
        bidx = np.zeros((8, N_RCH * RCH), np.int16)
        for b in range(8):
            mb = bnd == b
            n = int(mb.sum())
            gidx[b, 1:1 + n] = ix[mb]            # slot0 = pad
            buE[b, 1:1 + n] = buc[mb]
            bvE[b, 1:1 + n] = bvc[mb]
            # boundary entries: end-slot per row; entry r = cumcount(<= r-1?)
            cnt = np.bincount(rl[mb], minlength=ROWS_PER_CORE)
            ends = np.concatenate([[0], np.cumsum(cnt)])  # [12501]: e[r+1]=end(r)
            for k in range(N_RCH):
                r0 = k * (RCH - 1)               # rows [r0, r0+RCH-1)
                ent = ends[r0:r0 + RCH]
                if len(ent) < RCH:
                    ent = np.concatenate(
                        [ent, np.full(RCH - len(ent), ends[-1])])
                bidx[b, k * RCH:(k + 1) * RCH] = ent.astype(np.int16)

        # bueD [128, 4*Tb]: pair p cols [p*Tb,(p+1)Tb): rows 0-63 band 2p
        # (f,u) f-major; rows 64-127 band 2p+1
        bueD = np.zeros((P, 4 * Tb), np.float32)
        bveD = np.zeros((P, 4 * Tb), np.float32)
        for p in range(4):
            for half, b in ((0, 2 * p), (64, 2 * p + 1)):
                # (f,u): row half + f*4+u = bu[u]  (f-replicated)
                bueD[half:half + 64, p * Tb:(p + 1) * Tb] = np.tile(
                    buE[b].T, (F, 1))            # [4,Tb] tiled 16x -> 64 rows
                # (v,o): row half + v*16+o = bv[v] (o-replicated)
                bveD[half:half + 64, p * Tb:(p + 1) * Tb] = np.repeat(
                    bvE[b].T, F, axis=0)         # [4,Tb] repeat 16x
        d = dict(xsrc=xsrc, W2D=W2D, E32D=E32D, Fv2=Fv2, Fsum=Fsum,
                 gidx=wrap16(gidx, Tb), bidx=wrap16(bidx, N_RCH * RCH),
                 bueD=bueD.astype(bf16), bveD=bveD.astype(bf16))
        in_maps.append(d)
    return in_maps, Tb, n_chunks


def _build(Tb, n_chunks):
    from concourse import bacc, mybir
    import contextlib

    nc = bacc.Bacc(None, target_bir_lowering=False)
    dt = mybir.dt
    BP = N_RCH * RCH
    RC = RCH - 1                      # rows per boundary chunk (799)
    xsrcD = nc.dram_tensor("xsrc", [P, BLK], dt.float32, kind="ExternalInput")
    W2Dd = nc.dram_tensor("W2D", [P, 64], dt.bfloat16, kind="ExternalInput")
    E32Dd = nc.dram_tensor("E32D", [P, 128], dt.bfloat16, kind="ExternalInput")
    Fv2D = nc.dram_tensor("Fv2", [P, 32], dt.bfloat16, kind="ExternalInput")
    FsD = nc.dram_tensor("Fsum", [P, 16], dt.bfloat16, kind="ExternalInput")
    gidxD = nc.dram_tensor("gidx", [P, Tb // 16], dt.int16, kind="ExternalInput")
    bidxD = nc.dram_tensor("bidx", [P, BP // 16], dt.int16, kind="ExternalInput")
    bueDd = nc.dram_tensor("bueD", [P, 4 * Tb], dt.bfloat16, kind="ExternalInput")
    bveDd = nc.dram_tensor("bveD", [P, 4 * Tb], dt.bfloat16, kind="ExternalInput")
    outD = nc.dram_tensor("outD", [16, N_RCH * RC], dt.float32,
                          kind="ExternalOutput")

    with contextlib.ExitStack() as st:
        sb = lambda n, sh, t: st.enter_context(nc.sbuf_tensor(n, sh, t))
        ps = lambda n, sh: st.enter_context(nc.psum_tensor(n, sh, dt.float32))
        sem = lambda n: st.enter_context(nc.semaphore(n))

        xt = sb("xt", [P, BLK], dt.float32)
        Pt = sb("Pt", [P, Tb], dt.float32)
        git = sb("git", [P, Tb // 16], dt.int16)
        bit = sb("bit", [P, BP // 16], dt.int16)
        W2s = sb("W2s", [P, 64], dt.bfloat16)
        E32s = sb("E32s", [P, 128], dt.bfloat16)
        Fv2s = sb("Fv2s", [P, 32], dt.bfloat16)
        Fss = sb("Fss", [P, 16], dt.bfloat16)
        xj = sb("xj", [P, 2 * CH], dt.float32)
        xjb = sb("xjb", [32, 8 * CH], dt.bfloat16)
        Xb = [sb(f"Xb{p}", [P, 2 * CH], dt.bfloat16) for p in range(4)]
        bueS = sb("bueS", [P, 4 * CH], dt.bfloat16)
        bveS = sb("bveS", [P, 4 * CH], dt.bfloat16)
        ySBb = [sb(f"ySBb{p}", [P, CH], dt.bfloat16) for p in range(4)]
        msgS = sb("msgS", [P, CH], dt.float32)
        Bt = sb("Bt", [P, 2 * RCH], dt.float32)
        Dt = sb("Dt", [P, RC], dt.float32)
        Db = sb("Db", [P, RC], dt.bfloat16)
        Bo = sb("Bo", [16, 2 * RC], dt.float32)
        psA = [ps(f"psA{p}", [P, CH]) for p in range(4)]
        ps2 = [ps(f"ps2{p}", [P, CH]) for p in range(4)]

        s_ld = sem("s_ld")
        s_bl = sem("s_bl")
        s_xg = sem("s_xg")
        s_xjb = sem("s_xjb")
        s_Xp = sem("s_Xp")
        s_peA = sem("s_peA")
        s_pe2 = sem("s_pe2")
        s_y2 = sem("s_y2")
        s_fm = sem("s_fm")
        s_mcp = sem("s_mcp")
        s_cons = sem("s_cons")
        s_scan = sem("s_scan")
        s_bg = sem("s_bg")
        s_bdf = sem("s_bdf")
        s_bcv = sem("s_bcv")
        s_bfm = sem("s_bfm")
        s_out = sem("s_out")
        s_od = sem("s_od")

        po, ve, te, sy, ac = (nc.gpsimd, nc.vector, nc.tensor, nc.sync,
                              nc.scalar)

        # ---- resident loads ----
        for tdst, tsrc in ((xt, xsrcD), (git, gidxD), (bit, bidxD),
                           (W2s, W2Dd), (E32s, E32Dd), (Fv2s, Fv2D),
                           (Fss, FsD)):
            sy.dma_start(tdst[:], tsrc[:]).then_inc(s_ld, 16)
        NLD = 7 * 16

        # ---- stream loads (single-buffered per chunk) ----
        for g in range(n_chunks):
            if g > 0:
                sy.wait_ge(s_cons, g)
            for p in range(4):
                sy.dma_start(
                    bueS[:, p * CH:(p + 1) * CH],
                    bueDd[:, p * Tb + g * CH:p * Tb + (g + 1) * CH],
                ).then_inc(s_bl, 16)
                sy.dma_start(
                    bveS[:, p * CH:(p + 1) * CH],
                    bveDd[:, p * Tb + g * CH:p * Tb + (g + 1) * CH],
                ).then_inc(s_bl, 16)
        for k in range(N_RCH):
            sy.wait_ge(s_out, 2 * (k + 1))
            sy.dma_start(
                outD[:, k * RC:(k + 1) * RC],
                Bo[:, (k % 2) * RC:(k % 2) * RC + RC],
            ).then_inc(s_od, 16)
        sy.wait_ge(s_od, 16 * N_RCH)

        # ---- gpsimd ----
        po.wait_ge(s_ld, NLD)
        for g in range(n_chunks):
            if g >= 2:
                po.wait_ge(s_xjb, g - 1)
            po.ap_gather(
                out_ap=xj[:, (g % 2) * CH:(g % 2 + 1) * CH]
                .rearrange("p (n d) -> p n d", d=1),
                in_ap=xt[:].rearrange("p (n d) -> p n d", d=1),
                idxs_ap=git[:, g * (CH // 16):(g + 1) * (CH // 16)],
                channels=P, num_elems=BLK, d=1, num_idxs=CH,
            ).then_inc(s_xg, 1)
        po.wait_ge(s_cons, n_chunks)
        for k in range(N_RCH):
            if k >= 2:
                po.wait_ge(s_bdf, k - 1)   # ve diff freed Bt slot
            po.ap_gather(
                out_ap=Bt[:, (k % 2) * RCH:(k % 2 + 1) * RCH]
                .rearrange("p (n d) -> p n d", d=1),
                in_ap=Pt[:].rearrange("p (n d) -> p n d", d=1),
                idxs_ap=bit[:, k * (RCH // 16):(k + 1) * (RCH // 16)],
                channels=P, num_elems=Tb, d=1, num_idxs=RCH,
            ).then_inc(s_bg, 1)

        # ---- scalar (act): conversions + psum->sbuf moves ----
        for g in range(n_chunks):
            ac.wait_ge(s_xg, g + 1)
            if g >= 2:
                ac.wait_ge(s_peA, (g - 1) * 4)      # te consumed xjb g-2
            gc = (g % 2) * CH
            last = None
            for p in range(4):
                last = ac.copy(
                    out=xjb[0:32,
                            (p * 2 + g % 2) * CH:(p * 2 + g % 2 + 1) * CH],
                    in_=xj[32 * p:32 * (p + 1), gc:gc + CH])
            last.then_inc(s_xjb, 1)
            if g > 0:
                ac.wait_ge(s_cons, g)               # scan g-1 read msgS
            for p in range(4):
                ac.wait_ge(s_fm, g * 4 + p + 1)     # fold-mm done
                ac.copy(out=msgS[32 * p:32 * (p + 1), :],
                        in_=psA[p][0:32, :]).then_inc(s_mcp, 1)
        # boundary: Db = bf16(Dt); Bo strips from psum
        for k in range(N_RCH):
            ac.wait_ge(s_bdf, k + 1)
            if k > 0:
                ac.wait_ge(s_bfm, 2 * k)            # te done with Db k-1
            ac.copy(out=Db[:], in_=Dt[:]).then_inc(s_bcv, 1)
            bo = (k % 2) * RC
            if k >= 2:
                ac.wait_ge(s_od, 16 * (k - 1))      # dma freed Bo slot
            ac.wait_ge(s_bfm, 2 * k + 1)
            ac.copy(out=Bo[:, bo:bo + 512],
                    in_=psA[0][0:16, :]).then_inc(s_out, 1)
            ac.wait_ge(s_bfm, 2 * k + 2)
            ac.copy(out=Bo[:, bo + 512:bo + RC],
                    in_=psA[1][0:16, 0:RC - 512]).then_inc(s_out, 1)

        # ---- tensor engine ----
        te.wait_ge(s_ld, NLD)
        for g in range(n_chunks):
            te.wait_ge(s_xjb, g + 1)
            for p in range(4):
                if g > 0:
                    te.wait_ge(s_Xp, (g - 1) * 4 + p + 1)   # psA X read
                    te.wait_ge(s_mcp, (g - 1) * 4 + p + 1)  # psA msg copied
                te.matmul(
                    psA[p][:],
                    E32s[0:32, :],
                    xjb[0:32, (p * 2 + g % 2) * CH:(p * 2 + g % 2 + 1) * CH],
                    start=True, stop=True,
                ).then_inc(s_peA, 1)
            for p in range(4):
                te.wait_ge(s_Xp, g * 4 + p + 1)
                if g > 0:
                    te.wait_ge(s_y2, (g - 1) * 4 + p + 1)
                te.matmul(
                    ps2[p][0:64, :],
                    W2s[0:64, :],
                    Xb[p][0:64, (g % 2) * CH:(g % 2 + 1) * CH],
                    start=True, stop=True,
                )
                te.matmul(
                    ps2[p][64:128, :],
                    W2s[64:128, :],
                    Xb[p][64:128, (g % 2) * CH:(g % 2 + 1) * CH],
                    start=True, stop=True,
                ).then_inc(s_pe2, 1)
            for p in range(4):
                te.wait_ge(s_y2, g * 4 + p + 1)     # ySBb[p] ready
                te.matmul(
                    psA[p][0:32, :],
                    Fv2s[:, :],
                    ySBb[p][:],
                    start=True, stop=True,
                ).then_inc(s_fm, 1)
        # boundary fold matmuls
        for k in range(N_RCH):
            te.wait_ge(s_bcv, k + 1)
            if k > 0:
                te.wait_ge(s_out, 2 * k)            # Bo copies of k-1 done
            te.matmul(psA[0][0:16, :], Fss[:, :], Db[:, 0:512],
                      start=True, stop=True).then_inc(s_bfm, 1)
            te.matmul(psA[1][0:16, 0:RC - 512], Fss[:, :], Db[:, 512:RC],
                      start=True, stop=True).then_inc(s_bfm, 1)

        # ---- vector engine ----
        for g in range(n_chunks):
            ve.wait_ge(s_bl, (g + 1) * 128)
            for p in range(4):
                ve.wait_ge(s_peA, g * 4 + p + 1)
                ve.tensor_tensor(
                    out=Xb[p][:, (g % 2) * CH:(g % 2 + 1) * CH],
                    in0=psA[p][:],
                    in1=bueS[:, p * CH:(p + 1) * CH],
                    op=mybir.AluOpType.mult,
                ).then_inc(s_Xp, 1)
            for p in range(4):
                ve.wait_ge(s_pe2, g * 4 + p + 1)
                if g > 0:
                    ve.wait_ge(s_fm, (g - 1) * 4 + p + 1)   # te read ySBb
                ve.tensor_tensor(
                    out=ySBb[p][:],
                    in0=ps2[p][:],
                    in1=bveS[:, p * CH:(p + 1) * CH],
                    op=mybir.AluOpType.mult,
                ).then_inc(s_y2, 1)
            ve.wait_ge(s_mcp, (g + 1) * 4)          # msgS assembled
            col0 = g * CH
            init = 0.0 if col0 == 0 else Pt[:, col0 - 1:col0]
            last = ve.tensor_tensor_scan(
                out=Pt[:, col0:col0 + CH],
                data0=msgS[:], data1=msgS[:],
                initial=init,
                op0=mybir.AluOpType.add, op1=mybir.AluOpType.bypass)
            last.then_inc(s_cons, 1)
        for k in range(N_RCH):
            ve.wait_ge(s_bg, k + 1)
            if k > 0:
                ve.wait_ge(s_bcv, k)                # act read Dt of k-1
            bb = (k % 2) * RCH
            ve.tensor_tensor(
                out=Dt[:], in0=Bt[:, bb + 1:bb + RCH],
                in1=Bt[:, bb:bb + RCH - 1],
                op=mybir.AluOpType.subtract).then_inc(s_bdf, 1)
    nc.finalize()
    return nc


def kernel(x, edge_index, edge_attr, weight):
    from concourse.bass_utils import run_bass_kernel_spmd
    import os
    in_maps, Tb, n_chunks = _host_prep(x, edge_index, edge_attr, weight)
    nc = _build(Tb, n_chunks)
    trace = bool(os.environ.get("BASS_KERNEL_TRACE"))
    res = run_bass_kernel_spmd(nc, in_maps, core_ids=list(range(N_CORES)),
                               trace=trace)
    if trace and res.exec_time_ns is not None:
        print(f"HW exec time: {res.exec_time_ns} ns (mean {res.mean_exec_time_ns})")
    out = np.empty((N_NODES, F), np.float32)
    for c in range(N_CORES):
        o = res.results[c]["outD"]           # [16, N_RCH*(RCH-1)]
        out[c * ROWS_PER_CORE:(c + 1) * ROWS_PER_CORE] = \
            o[:, :ROWS_PER_CORE].T
    return out
